# revision 18
# baseline (speedup 1.0000x reference)
"""BERT self-attention (B=8, S=1024, D=768, H=12) on 8 TRN2 NeuronCores.

Sharding: batch across the 8 cores (one batch element per core).

Per-core dataflow (all matmuls bf16 on the tensor engine, fp32 PSUM):
  - host pre-transposes hs[b] -> hsT [D, S] and the weights -> W.T [D, D]
    so the contraction dim (din) lands on SBUF partitions; 1/sqrt(dh) is
    folded into Wk/bk on the host.
  - qT[dout, s], kT[dout, s] = W.T-tiles (stationary) x hsT (moving).
    qT2/kT2 are partition-swapped copies (halves exchanged, via
    SBUF->SBUF DMA): exact for the K=64 head contraction since the sum
    reindexes, and they let consecutive j score matmuls land in distinct
    PE row groups (auto tile_position from base_partition) so two run
    concurrently -> scores cost halves.
  - v[s, dout] = hsT-tiles (stationary) x Wv.T (moving), stored head-major
    [s, (h, 65)] with a ones column per head (denominator accumulator).
  - per head, per j-pair: scoresT[ks, qs] for j0 (rows r0 of qT/kT) and
    j1 (rows r1 of qT2/kT2) packed concurrently; exp is a single ACT op
    per j that folds the click gate and additive mask exactly:
    exp(click[ks]*s + mask[ks]) via per-partition scale/bias APs.
    ctxT[65, qs] accumulates v_aug.T @ expT over j; row 64 = denominator.
  - normalize per head: reciprocal_approx_fast on the denominator row,
    PE partition-broadcast (ones stationary), multiply, DMA out bf16;
    finishers are paced ~1 head later so nothing stalls the PE queue.
  - host transposes back and upcasts on unshard.
"""

import sys

sys.path.insert(0, "/opt/trn_rl_repo")

import numpy as np

B, S, D, H = 8, 1024, 768, 12
DH = D // H  # 64
NT = D // 128  # 6 dout/din tiles
NS = S // 128  # 8 s tiles
QS = 512  # qs chunk (PSUM bank / fp32 moving max)
NWARM = 22  # PE warmup matmuls ([128,512] each) to cover startup DMA

_built = None


def _apply_workarounds():
    """Container fixes: (1) walrus here accepts at most one sync wait on the
    Tile tail Drain -> split extra waits onto SP nops; (2) antenv.axon_hooks
    is missing from the image (needed only for trace=True profiling)."""
    import os

    import concourse.tile as tile
    from concourse.vector_clock import ScopedClock

    if getattr(tile.TileContext, "_drain_split_patched", False):
        return

    def _drain_and_barrier(self, tick_clock, wait_clock):
        drain_inst = self.nc.sync.drain()
        wait_clock.add_sem_waits(
            drain_inst.ins, ScopedClock({None: tick_clock.global_clock})
        )
        si = drain_inst.ins.sync_info
        if si is not None and len(si.on_wait) > 1:
            waits = list(si.on_wait)
            si.on_wait = waits[:1]
            for w in waits[1:]:
                nop = self.nc.sync.nop(nofuse=True, hint="drain_wait_split")
                nsi = nop.ins.sync_info
                if nsi is None:
                    import bass_rust

                    nop.ins.sync_info = bass_rust.SyncInfo(on_update=[], on_wait=[w])
                else:
                    nsi.on_wait = [w]

        self.nc.all_engine_barrier()
        assert self.sems is not None
        popped = self.nc._tile_sem_poison_stack.pop()
        assert popped is self._sem_poison
        self.nc.clear_and_free_semaphores(list(self.sems.allocated().values()))
        self.nc.all_engine_barrier()

    tile.TileContext._drain_and_barrier = _drain_and_barrier
    tile.TileContext._drain_split_patched = True

    hooks_src = (
        "_axon_ntff_profile_hook = None\n\n\n"
        "def set_axon_ntff_profile_hook(hook):\n"
        "    global _axon_ntff_profile_hook\n"
        "    _axon_ntff_profile_hook = hook\n\n\n"
        "def get_axon_ntff_profile_hook():\n"
        "    return _axon_ntff_profile_hook\n"
    )
    for d in ("/root/.axon_site/_ro/trn_rl_repo/antenv", "/opt/trn_rl_repo/antenv"):
        path = os.path.join(d, "axon_hooks.py")
        try:
            if os.path.isdir(d) and not os.path.exists(path):
                with open(path, "w") as f:
                    f.write(hooks_src)
        except OSError:
            pass


def _build():
    import concourse.bass as bass
    import concourse.tile as tile
    from concourse import mybir

    f32 = mybir.dt.float32
    f32r = mybir.dt.float32r
    bf16 = mybir.dt.bfloat16
    Exp = mybir.ActivationFunctionType.Exp
    mult = mybir.AluOpType.mult

    nc = bass.Bass()
    hsT_d = nc.dram_tensor("hsT", [D, S], bf16, kind="ExternalInput")
    wT_d = {
        w: nc.dram_tensor(f"w{w}T", [D, D], bf16, kind="ExternalInput")
        for w in ("q", "k", "v")
    }
    bqT_d = nc.dram_tensor("bqT", [128, NT], f32, kind="ExternalInput")
    bkT_d = nc.dram_tensor("bkT", [128, NT], f32, kind="ExternalInput")
    bvB_d = nc.dram_tensor("bvB", [128, D], f32, kind="ExternalInput")
    clickT_d = nc.dram_tensor("clickT", [128, NS], f32, kind="ExternalInput")
    maskT_d = nc.dram_tensor("maskT", [128, NS], f32, kind="ExternalInput")
    ones97_d = nc.dram_tensor("ones97", [97, DH], f32r, kind="ExternalInput")
    vones_d = nc.dram_tensor("vones", [128, NS, H], bf16, kind="ExternalInput")
    perm_d = nc.dram_tensor("perm", [128, 128], bf16, kind="ExternalInput")
    out_d = nc.dram_tensor("out", [H, DH, S], bf16, kind="ExternalOutput")

    with tile.TileContext(nc) as tc:
        from contextlib import ExitStack

        with ExitStack() as ctx:
            consts = ctx.enter_context(tc.tile_pool(name="consts", bufs=1))
            big = ctx.enter_context(tc.tile_pool(name="big", bufs=1))
            exps = ctx.enter_context(tc.tile_pool(name="exps", bufs=4))
            fin = ctx.enter_context(tc.tile_pool(name="fin", bufs=2))
            # PSUM: scores 2x[128,1024]f32 (4 banks) + ctx [65,1024] (2) +
            # proj/bc 2x[128,512] (2) = 8 banks exactly.
            psc = ctx.enter_context(tc.tile_pool(name="psc", bufs=2, space="PSUM"))
            pcx = ctx.enter_context(tc.tile_pool(name="pcx", bufs=1, space="PSUM"))
            pp = ctx.enter_context(tc.tile_pool(name="pp", bufs=2, space="PSUM"))

            # ---- constants ----
            bqT = consts.tile([128, NT], f32)
            nc.sync.dma_start(out=bqT, in_=bqT_d[:])
            bkT = consts.tile([128, NT], f32)
            nc.sync.dma_start(out=bkT, in_=bkT_d[:])
            bvB = consts.tile([128, D], f32)
            nc.sync.dma_start(out=bvB, in_=bvB_d[:])
            clickT = consts.tile([128, NS], f32)
            nc.sync.dma_start(out=clickT, in_=clickT_d[:])
            maskT = consts.tile([128, NS], f32)
            nc.sync.dma_start(out=maskT, in_=maskT_d[:])
            ones97 = consts.tile([97, DH], f32r)
            nc.sync.dma_start(out=ones97, in_=ones97_d[:])
            perm = consts.tile([128, 128], bf16)
            nc.sync.dma_start(out=perm, in_=perm_d[:])

            # ---- inputs, split per k-tile so the first matmuls start early ----
            hsT = big.tile([128, NT, S], bf16)
            wT = {}
            for w in ("q", "k", "v"):
                wT[w] = big.tile([128, NT, D], bf16, tag=f"w{w}", name=f"w{w}sb")
            hsT_r = hsT_d.rearrange("(t p) s -> p t s", p=128)
            wT_r = {w: wT_d[w].rearrange("(t p) d -> p t d", p=128) for w in wT_d}
            nc.sync.dma_start(out=hsT, in_=hsT_r)
            for w in ("q", "k", "v"):
                nc.sync.dma_start(out=wT[w], in_=wT_r[w])

            warm = consts.tile([128, QS], bf16, name="warm")
            nc.vector.memset(warm, 0.0)
            # preload the exp activation table set during the DMA phase
            pre = consts.tile([128, 1], f32, name="pre")
            nc.scalar.activation(pre, warm[:, 0:1], Exp)
            for wi in range(NWARM):
                wp = pp.tile([128, QS], f32, tag="proj", name=f"warm{wi}")
                nc.tensor.matmul(wp, warm[:, 0:128], warm, start=True, stop=True)

            qT = big.tile([128, NT, S], bf16, tag="qT")
            kT = big.tile([128, NT, S], bf16, tag="kT")
            qT2 = big.tile([128, NT, S], bf16, tag="qT2")
            kT2 = big.tile([128, NT, S], bf16, tag="kT2")
            # v_aug: [s_partition, s_tile, head-major (h, dh | ones)]
            v = big.tile([128, NS, H * (DH + 1)], bf16, tag="v")
            dens = big.tile([97, S], f32, tag="dens")
            nc.vector.memset(dens, 1.0)
            densf = big.tile([97, S], f32, tag="densf")
            densr = big.tile([97, S], f32r, tag="densr")

            def qk_chunk(w, dest, bias, t, c):
                cs = slice(c * QS, (c + 1) * QS)
                ps = pp.tile([128, QS], f32, tag="proj")
                for k in range(NT):
                    nc.tensor.matmul(
                        ps,
                        wT[w][:, k, t * 128 : (t + 1) * 128],
                        hsT[:, k, cs],
                        start=(k == 0),
                        stop=(k == NT - 1),
                    )
                nc.vector.tensor_scalar_add(dest[:, t, cs], ps, bias[:, t : t + 1])

            def qk_swap(dest, dest2, t):
                # partition-halves rotation via a 0/1 permutation matmul
                # (exact in bf16): head data relocates to the other PE row
                # group so consecutive-j score matmuls run concurrently.
                for c in range(S // QS):
                    cs = slice(c * QS, (c + 1) * QS)
                    ps = pp.tile([128, QS], f32, tag="proj")
                    nc.tensor.matmul(ps, perm, dest[:, t, cs], start=True, stop=True)
                    nc.vector.tensor_copy(dest2[:, t, cs], ps)

            def proj_v(si):
                """v rows for s-tile si, head-major with ones col."""
                vsi = v[:, si, :].rearrange("p (h e) -> p h e", e=DH + 1)
                for c0, cn in ((0, 512), (512, 256)):
                    h0, nh = c0 // DH, cn // DH
                    ps = pp.tile([128, cn], f32, tag="proj")
                    for k in range(NT):
                        nc.tensor.matmul(
                            ps,
                            hsT[:, k, si * 128 : (si + 1) * 128],
                            wT["v"][:, k, c0 : c0 + cn],
                            start=(k == 0),
                            stop=(k == NT - 1),
                        )
                    nc.vector.tensor_tensor(
                        out=vsi[:, h0 : h0 + nh, 0:DH],
                        in0=ps.rearrange("p (h e) -> p h e", e=DH),
                        in1=bvB[:, c0 : c0 + cn].rearrange("p (h e) -> p h e", e=DH),
                        op=mybir.AluOpType.add,
                    )
                nc.sync.dma_start(out=vsi[:, :, DH : DH + 1], in_=vones_d[:, si, :])

            def attn_head(h, filler):
                """One head; j-pairs packed into both PE row groups."""
                t = h // 2
                lo, hi = slice(0, 64), slice(64, 128)
                r0 = hi if h % 2 else lo  # head's rows in qT/kT
                r1 = lo if h % 2 else hi  # head's rows in qT2/kT2
                ctx_ps = pcx.tile([DH + 1, S], f32, tag="ctx", name=f"ctx{h}")
                for jj in range(NS // 2):
                    j0, j1 = 2 * jj, 2 * jj + 1
                    sc0 = psc.tile([128, S], f32, tag="sc")
                    sc1 = psc.tile([128, S], f32, tag="sc")
                    for c in range(S // QS):
                        cs = slice(c * QS, (c + 1) * QS)
                        nc.tensor.matmul(
                            sc0[:, cs],
                            kT[r0, t, j0 * 128 : (j0 + 1) * 128],
                            qT[r0, t, cs],
                            start=True,
                            stop=True,
                        )
                        nc.tensor.matmul(
                            sc1[:, cs],
                            kT2[r1, t, j1 * 128 : (j1 + 1) * 128],
                            qT2[r1, t, cs],
                            start=True,
                            stop=True,
                        )
                    ets = []
                    for j, sc in ((j0, sc0), (j1, sc1)):
                        et = exps.tile([128, S], bf16, tag="exp")
                        nc.scalar.activation(
                            et,
                            sc,
                            Exp,
                            bias=maskT[:, j : j + 1],
                            scale=clickT[:, j : j + 1],
                        )
                        ets.append(et)
                    filler(jj)
                    for j, et in ((j0, ets[0]), (j1, ets[1])):
                        va = v[:, j, :].rearrange("p (h e) -> p h e", e=DH + 1)[:, h, :]
                        for c in range(S // QS):
                            cs = slice(c * QS, (c + 1) * QS)
                            nc.tensor.matmul(
                                ctx_ps[:, cs],
                                va,
                                et[:, cs],
                                start=(j == 0),
                                stop=(j == NS - 1),
                            )
                cs_sb = fin.tile(
                    [DH + 1, S], bf16, tag="ctx_sb", name=f"cs{h}", bufs=6
                )
                nc.vector.tensor_copy(cs_sb, ctx_ps)
                hh = 32 * (h % 4)
                nc.vector.tensor_copy(dens[hh : hh + 1, :], cs_sb[DH : DH + 1, :])
                return cs_sb

            def act_recip(lo, hi):
                """1/d = exp(-ln d) on the ACT engine for dens rows [lo, hi).
                DVE's iterative-divide reciprocal (~6.5 cyc/elem) would block
                its strict-FIFO queue ~6.6us and stall PSUM evacuations; two
                ACT ops cost ~2.2us in ACT slack (same table set as exp)."""
                nc.scalar.activation(
                    densf[lo:hi, :], dens[lo:hi, :], mybir.ActivationFunctionType.Ln
                )
                nc.scalar.activation(densr[lo:hi, :], densf[lo:hi, :], Exp, scale=-1.0)

            def fin_head(h, cs_sb):
                """PE broadcast of 1/denominator + normalize + store (paced
                ~1 head after the reciprocal so nothing stalls the PE queue)."""
                hh = 32 * (h % 4)
                for c in range(S // QS):
                    cs = slice(c * QS, (c + 1) * QS)
                    bc = pp.tile([DH, QS], f32, tag="proj", name=f"bc{h}_{c}")
                    nc.tensor.matmul(
                        bc,
                        ones97[hh : hh + 1, :],
                        densr[hh : hh + 1, cs],
                        start=True,
                        stop=True,
                        tile_position=(hh, 0),
                    )
                    nc.vector.tensor_tensor(
                        out=cs_sb[0:DH, cs], in0=cs_sb[0:DH, cs], in1=bc, op=mult
                    )
                nc.sync.dma_start(out=out_d[h], in_=cs_sb[0:DH, :])

            # ---- emission schedule ----
            # qk tile 0 + swaps up front; v folded into head 0's slots;
            # qk tile t (2..5) split across heads 2t-2, 2t-1; swaps emitted
            # once both chunks of a (w, t) are done; per-head normalize
            # finishers paced one head later.
            for c in range(2):
                qk_chunk("q", qT, bqT, 0, c)
            for c in range(2):
                qk_chunk("k", kT, bkT, 0, c)
            qk_swap(qT, qT2, 0)
            qk_swap(kT, kT2, 0)

            fillers = {h: {} for h in range(H)}
            fillers[0] = {
                jj: [lambda a=2 * jj, b=2 * jj + 1: (proj_v(a), proj_v(b))]
                for jj in range(4)
            }
            fillers[1] = {
                0: [lambda: qk_chunk("q", qT, bqT, 1, 0)],
                1: [lambda: qk_chunk("q", qT, bqT, 1, 1)],
                2: [
                    lambda: qk_chunk("k", kT, bkT, 1, 0),
                    lambda: qk_swap(qT, qT2, 1),
                ],
                3: [
                    lambda: qk_chunk("k", kT, bkT, 1, 1),
                    lambda: qk_swap(kT, kT2, 1),
                ],
            }
            for h in range(2, 10):
                t = h // 2 + 1
                w, dest, bias = ("q", qT, bqT) if h % 2 == 0 else ("k", kT, bkT)
                dest2 = qT2 if h % 2 == 0 else kT2
                fillers[h] = {
                    0: [lambda w=w, dest=dest, bias=bias, t=t: qk_chunk(w, dest, bias, t, 0)],
                    2: [lambda w=w, dest=dest, bias=bias, t=t: qk_chunk(w, dest, bias, t, 1)],
                    3: [lambda dest=dest, dest2=dest2, t=t: qk_swap(dest, dest2, t)],
                }

            # normalize pacing: a group's reciprocal lands at the next
            # group's first head; finishers one head later still, so the
            # PE-queue bc matmuls never wait on an in-flight reciprocal.
            done = {}
            sched = {
                4: {0: [lambda: act_recip(0, 97)]},
                5: {0: [lambda: fin_head(0, done[0])], 2: [lambda: fin_head(1, done[1])]},
                6: {0: [lambda: fin_head(2, done[2])], 2: [lambda: fin_head(3, done[3])]},
                8: {0: [lambda: act_recip(0, 97)]},
                9: {0: [lambda: fin_head(4, done[4])], 2: [lambda: fin_head(5, done[5])]},
                10: {
                    0: [lambda: fin_head(6, done[6])],
                    1: [lambda: act_recip(0, 33)],
                    2: [lambda: fin_head(7, done[7])],
                },
                11: {0: [lambda: fin_head(8, done[8])], 2: [lambda: fin_head(9, done[9])]},
            }
            for h in range(H):
                hf = fillers[h]
                extra = sched.get(h, {})

                def fill(jj, hf=hf, extra=extra):
                    for fn in hf.get(jj, []):
                        fn()
                    for fn in extra.get(jj, []):
                        fn()

                done[h] = attn_head(h, fill)
            act_recip(64, 97)
            fin_head(10, done[10])
            fin_head(11, done[11])

    _install_multiwait_split(nc)
    return nc


def _install_multiwait_split(nc):
    """This walrus build accepts at most one sync wait per instruction
    (Drain/CTRL and Matmult/LDWEIGHTS structs at least). Tile attaches
    several. Split extras onto single-wait NoOps inserted just before the
    instruction, at JSON-serialization time so every compile path sees it."""
    import types

    import orjson
    from concourse import mybir

    def to_json_bytes(self):
        m = orjson.loads(mybir.module_to_json_bytes(self.m))
        n = 0
        for fn in m.get("functions", []):
            for bb in fn.get("blocks", []):
                insts = bb.get("instructions", [])
                out = []
                for inst in insts:
                    si = inst.get("sync_info")
                    waits = (si or {}).get("on_wait") or []
                    if len(waits) > 1:
                        for w in waits[:-1]:
                            n += 1
                            out.append(
                                {
                                    "debug": inst.get("debug", 0),
                                    "engine": inst["engine"],
                                    "ins": [],
                                    "name": f"I-mws{n}",
                                    "opcode": "NoOp",
                                    "outs": [],
                                    "sync_info": {"on_update": [], "on_wait": [w]},
                                    "text_hint": "multiwait_split",
                                }
                            )
                        si["on_wait"] = [waits[-1]]
                    out.append(inst)
                bb["instructions"] = out
        return orjson.dumps(m)

    nc.to_json_bytes = types.MethodType(to_json_bytes, nc)


def _get_built():
    global _built
    if _built is None:
        _apply_workarounds()
        _built = _build()
    return _built


def _prep_in_maps(inputs):
    hs = np.asarray(inputs["hidden_states"], np.float32)
    mask = np.asarray(inputs["attention_mask"], np.float32)
    click = np.asarray(inputs["click_times"], np.float32)
    Wq = np.asarray(inputs["Wq"], np.float32)
    bq = np.asarray(inputs["bq"], np.float32)
    Wk = np.asarray(inputs["Wk"], np.float32)
    bk = np.asarray(inputs["bk"], np.float32)
    Wv = np.asarray(inputs["Wv"], np.float32)
    bv = np.asarray(inputs["bv"], np.float32)

    import ml_dtypes

    bf16 = ml_dtypes.bfloat16
    scale = 1.0 / np.sqrt(np.float32(DH))
    shared = {
        "wqT": np.ascontiguousarray(Wq.T).astype(bf16),
        "wkT": np.ascontiguousarray(Wk.T * scale).astype(bf16),
        "wvT": np.ascontiguousarray(Wv.T).astype(bf16),
        "bqT": np.ascontiguousarray(bq.reshape(NT, 128).T),
        "bkT": np.ascontiguousarray((bk * scale).reshape(NT, 128).T),
        "bvB": np.ascontiguousarray(np.broadcast_to(bv, (128, D))),
        "ones97": np.ones((97, DH), np.float32),
        "vones": np.ones((128, NS, H), bf16),
        "perm": np.eye(128, dtype=np.float32)[:, list(range(64, 128)) + list(range(64))].astype(bf16),
    }
    in_maps = []
    for b in range(B):
        m = dict(shared)
        m["hsT"] = np.ascontiguousarray(hs[b].T).astype(bf16)
        m["clickT"] = np.ascontiguousarray(click[b].reshape(NS, 128).T)
        m["maskT"] = np.ascontiguousarray(mask[b, 0, 0].reshape(NS, 128).T)
        in_maps.append(m)
    return in_maps


def run(inputs, trace=False, tmpdir=None):
    """Run on the 8 cores; returns (output [B,S,D], BassKernelResults)."""
    from concourse.bass_utils import run_bass_kernel_spmd

    nc = _get_built()
    in_maps = _prep_in_maps(inputs)
    res = run_bass_kernel_spmd(
        nc, in_maps, list(range(B)), trace=trace, tmpdir=tmpdir
    )
    out = np.empty((B, S, D), np.float32)
    for b in range(B):
        ctxT = np.asarray(res.results[b]["out"], dtype=np.float32)  # [H, DH, S]
        out[b] = ctxT.transpose(2, 0, 1).reshape(S, D)
    return out, res


def kernel(**inputs) -> np.ndarray:
    out, _ = run(inputs)
    return out


# revision 20
# speedup vs baseline: 1.2068x; 1.2068x over previous
"""BERT self-attention (B=8, S=1024, D=768, H=12) on 8 TRN2 NeuronCores.

Sharding: batch across the 8 cores (one batch element per core).

Per-core dataflow (all matmuls bf16 on the tensor engine, fp32 PSUM):
  - host pre-transposes hs[b] -> hsT [D, S] and weights -> W.T; 1/sqrt(dh)
    folds into Wk/bk; the click gate folds into a second moving operand
    hsTk = hsT * click[s] used only by the k-projection; exp(mask) folds
    into the v rows (and their ones column), all host-side or free.
  - qT/kT[dout, s] = W.T-tiles (stationary) x hsT/hsTk (moving).
    qT2/kT2 are partition-half-rotated copies made with a 0/1 permutation
    matmul (exact; sum over partitions reindexes), so consecutive-j score
    matmuls land in distinct PE row groups and run concurrently.
  - v[s, dout] head-major [s, (h, 65)] with a ones column (denominator).
  - per head, per j-pair jj: scoresT for j0 (rows r0 of qT/kT) and j1
    (rows r1 of qT2/kT2) packed concurrently; one plain Exp per j
    (PSUM->SBUF bf16); ctx matmuls are software-pipelined ONE jj-slot
    behind (and across heads) so the in-order PE queue never blocks on
    an exp that hasn't finished; ctxT[65, qs] accumulates over j.
  - normalize: 1/denominator = exp(-ln d) on the ACT engine (same table
    set as exp; DVE's iterative reciprocal would block its FIFO ~6.6us),
    batched 4 heads into partition rows; PE partition-broadcast (ones
    stationary at the head's row), multiply, DMA out bf16.
  - host transposes back and upcasts on unshard.
"""

import sys

sys.path.insert(0, "/opt/trn_rl_repo")

import numpy as np

B, S, D, H = 8, 1024, 768, 12
DH = D // H  # 64
NT = D // 128  # 6 dout/din tiles
NS = S // 128  # 8 s tiles
QS = 512  # qs chunk (PSUM bank / fp32 moving max)
NWARM = 22  # PE warmup matmuls ([128,512] each) to cover startup DMA

_built = None


def _apply_workarounds():
    """Container fixes: (1) walrus here accepts at most one sync wait on the
    Tile tail Drain -> split extra waits onto SP nops; (2) antenv.axon_hooks
    is missing from the image (needed only for trace=True profiling)."""
    import os

    import concourse.tile as tile
    from concourse.vector_clock import ScopedClock

    if getattr(tile.TileContext, "_drain_split_patched", False):
        return

    def _drain_and_barrier(self, tick_clock, wait_clock):
        drain_inst = self.nc.sync.drain()
        wait_clock.add_sem_waits(
            drain_inst.ins, ScopedClock({None: tick_clock.global_clock})
        )
        si = drain_inst.ins.sync_info
        if si is not None and len(si.on_wait) > 1:
            waits = list(si.on_wait)
            si.on_wait = waits[:1]
            for w in waits[1:]:
                nop = self.nc.sync.nop(nofuse=True, hint="drain_wait_split")
                nsi = nop.ins.sync_info
                if nsi is None:
                    import bass_rust

                    nop.ins.sync_info = bass_rust.SyncInfo(on_update=[], on_wait=[w])
                else:
                    nsi.on_wait = [w]

        self.nc.all_engine_barrier()
        assert self.sems is not None
        popped = self.nc._tile_sem_poison_stack.pop()
        assert popped is self._sem_poison
        self.nc.clear_and_free_semaphores(list(self.sems.allocated().values()))
        self.nc.all_engine_barrier()

    tile.TileContext._drain_and_barrier = _drain_and_barrier
    tile.TileContext._drain_split_patched = True

    hooks_src = (
        "_axon_ntff_profile_hook = None\n\n\n"
        "def set_axon_ntff_profile_hook(hook):\n"
        "    global _axon_ntff_profile_hook\n"
        "    _axon_ntff_profile_hook = hook\n\n\n"
        "def get_axon_ntff_profile_hook():\n"
        "    return _axon_ntff_profile_hook\n"
    )
    for d in ("/root/.axon_site/_ro/trn_rl_repo/antenv", "/opt/trn_rl_repo/antenv"):
        path = os.path.join(d, "axon_hooks.py")
        try:
            if os.path.isdir(d) and not os.path.exists(path):
                with open(path, "w") as f:
                    f.write(hooks_src)
        except OSError:
            pass


def _build():
    import concourse.bass as bass
    import concourse.tile as tile
    from concourse import mybir

    f32 = mybir.dt.float32
    f32r = mybir.dt.float32r
    bf16 = mybir.dt.bfloat16
    Exp = mybir.ActivationFunctionType.Exp
    Ln = mybir.ActivationFunctionType.Ln
    mult = mybir.AluOpType.mult

    nc = bass.Bass()
    hsT_d = nc.dram_tensor("hsT", [D, S], bf16, kind="ExternalInput")
    hsTk_d = nc.dram_tensor("hsTk", [D, S], bf16, kind="ExternalInput")
    wT_d = {
        w: nc.dram_tensor(f"w{w}T", [D, D], bf16, kind="ExternalInput")
        for w in ("q", "k", "v")
    }
    bqT_d = nc.dram_tensor("bqT", [128, NT], f32, kind="ExternalInput")
    bkT_d = nc.dram_tensor("bkT", [128, NT], f32, kind="ExternalInput")
    bvB_d = nc.dram_tensor("bvB", [128, D], f32, kind="ExternalInput")
    maskT_d = nc.dram_tensor("maskT", [128, NS], f32, kind="ExternalInput")
    ones97_d = nc.dram_tensor("ones97", [97, DH], f32r, kind="ExternalInput")
    vones_d = nc.dram_tensor("vones", [128, NS, H], bf16, kind="ExternalInput")
    perm_d = nc.dram_tensor("perm", [128, 128], bf16, kind="ExternalInput")
    out_d = nc.dram_tensor("out", [H, DH, S], bf16, kind="ExternalOutput")

    with tile.TileContext(nc) as tc:
        from contextlib import ExitStack

        with ExitStack() as ctx:
            consts = ctx.enter_context(tc.tile_pool(name="consts", bufs=1))
            big = ctx.enter_context(tc.tile_pool(name="big", bufs=1))
            exps = ctx.enter_context(tc.tile_pool(name="exps", bufs=5))
            fin = ctx.enter_context(tc.tile_pool(name="fin", bufs=2))
            # PSUM: scores 2x[128,1024]f32 (4 banks) + ctx [65,1024] (2) +
            # proj/bc 2x[128,512] (2) = 8 banks exactly.
            psc = ctx.enter_context(tc.tile_pool(name="psc", bufs=2, space="PSUM"))
            pcx = ctx.enter_context(tc.tile_pool(name="pcx", bufs=1, space="PSUM"))
            pp = ctx.enter_context(tc.tile_pool(name="pp", bufs=2, space="PSUM"))

            # ---- big inputs first (their transfers gate the first matmuls;
            # consts are consumed later) ----
            hsT = big.tile([128, NT, S], bf16)
            hsTk = big.tile([128, NT, S], bf16, tag="hsTk")
            wT = {}
            for w in ("q", "k", "v"):
                wT[w] = big.tile([128, NT, D], bf16, tag=f"w{w}", name=f"w{w}sb")
            nc.sync.dma_start(out=hsT, in_=hsT_d.rearrange("(t p) s -> p t s", p=128))
            nc.sync.dma_start(
                out=wT["q"], in_=wT_d["q"].rearrange("(t p) d -> p t d", p=128)
            )
            nc.sync.dma_start(
                out=hsTk, in_=hsTk_d.rearrange("(t p) s -> p t s", p=128)
            )
            for w in ("k", "v"):
                nc.sync.dma_start(
                    out=wT[w], in_=wT_d[w].rearrange("(t p) d -> p t d", p=128)
                )

            # ---- constants ----
            bqT = consts.tile([128, NT], f32)
            nc.sync.dma_start(out=bqT, in_=bqT_d[:])
            bkT = consts.tile([128, NT], f32)
            nc.sync.dma_start(out=bkT, in_=bkT_d[:])
            bvB = consts.tile([128, D], f32)
            nc.sync.dma_start(out=bvB, in_=bvB_d[:])
            maskT = consts.tile([128, NS], f32)
            nc.sync.dma_start(out=maskT, in_=maskT_d[:])
            ones97 = consts.tile([97, DH], f32r)
            nc.sync.dma_start(out=ones97, in_=ones97_d[:])
            perm = consts.tile([128, 128], bf16)
            nc.sync.dma_start(out=perm, in_=perm_d[:])

            warm = consts.tile([128, QS], bf16, name="warm")
            nc.vector.memset(warm, 0.0)
            # preload the exp/ln activation table set during the DMA phase
            pre = consts.tile([128, 1], f32, name="pre")
            nc.scalar.activation(pre, warm[:, 0:1], Exp)
            for wi in range(NWARM):
                wp = pp.tile([128, QS], f32, tag="proj", name=f"warm{wi}")
                nc.tensor.matmul(wp, warm[:, 0:128], warm, start=True, stop=True)

            qT = big.tile([128, NT, S], bf16, tag="qT")
            kT = big.tile([128, NT, S], bf16, tag="kT")
            qT2 = big.tile([128, NT, S], bf16, tag="qT2")
            kT2 = big.tile([128, NT, S], bf16, tag="kT2")
            # v_aug: [s_partition, s_tile, head-major (h, dh | ones)]
            v = big.tile([128, NS, H * (DH + 1)], bf16, tag="v")
            dens = big.tile([97, S], f32, tag="dens")
            nc.vector.memset(dens, 1.0)
            densf = big.tile([97, S], f32, tag="densf")
            densr = big.tile([97, S], f32r, tag="densr")

            def qk_chunk(w, src, dest, bias, t, c):
                cs = slice(c * QS, (c + 1) * QS)
                ps = pp.tile([128, QS], f32, tag="proj")
                for k in range(NT):
                    nc.tensor.matmul(
                        ps,
                        wT[w][:, k, t * 128 : (t + 1) * 128],
                        src[:, k, cs],
                        start=(k == 0),
                        stop=(k == NT - 1),
                    )
                nc.vector.tensor_scalar_add(dest[:, t, cs], ps, bias[:, t : t + 1])

            def qk_swap(dest, dest2, t):
                # partition-halves rotation via a 0/1 permutation matmul
                # (exact in bf16): head data relocates to the other PE row
                # group so consecutive-j score matmuls run concurrently.
                for c in range(S // QS):
                    cs = slice(c * QS, (c + 1) * QS)
                    ps = pp.tile([128, QS], f32, tag="proj")
                    nc.tensor.matmul(ps, perm, dest[:, t, cs], start=True, stop=True)
                    nc.vector.tensor_copy(dest2[:, t, cs], ps)

            def proj_v(si):
                """v rows for s-tile si, head-major with ones col, scaled by
                exp(mask) (folds the additive mask through the softmax)."""
                vsi = v[:, si, :].rearrange("p (h e) -> p h e", e=DH + 1)
                for c0, cn in ((0, 512), (512, 256)):
                    h0, nh = c0 // DH, cn // DH
                    ps = pp.tile([128, cn], f32, tag="proj")
                    for k in range(NT):
                        nc.tensor.matmul(
                            ps,
                            hsT[:, k, si * 128 : (si + 1) * 128],
                            wT["v"][:, k, c0 : c0 + cn],
                            start=(k == 0),
                            stop=(k == NT - 1),
                        )
                    nc.vector.tensor_tensor(
                        out=vsi[:, h0 : h0 + nh, 0:DH],
                        in0=ps.rearrange("p (h e) -> p h e", e=DH),
                        in1=bvB[:, c0 : c0 + cn].rearrange("p (h e) -> p h e", e=DH),
                        op=mybir.AluOpType.add,
                    )
                nc.sync.dma_start(out=vsi[:, :, DH : DH + 1], in_=vones_d[:, si, :])
                nc.vector.tensor_scalar_mul(
                    v[:, si, :], v[:, si, :], maskT[:, si : si + 1]
                )

            def attn_head(h, filler, prev_tail):
                """One head. ctx matmuls run one jj-slot behind the scores
                (software pipeline) so the in-order PE queue never parks on
                an exp still in flight; the previous head's last ctx + PSUM
                evacuation are emitted inside this head's slot 0."""
                t = h // 2
                lo, hi = slice(0, 64), slice(64, 128)
                r0 = hi if h % 2 else lo  # head's rows in qT/kT
                r1 = lo if h % 2 else hi  # head's rows in qT2/kT2
                ctx_ps = pcx.tile([DH + 1, S], f32, tag="ctx", name=f"ctx{h}")

                def do_ctx(pend):
                    for j, et in pend:
                        va = v[:, j, :].rearrange("p (h e) -> p h e", e=DH + 1)[
                            :, h, :
                        ]
                        for c in range(S // QS):
                            cs = slice(c * QS, (c + 1) * QS)
                            nc.tensor.matmul(
                                ctx_ps[:, cs],
                                va,
                                et[:, cs],
                                start=(j == 0),
                                stop=(j == NS - 1),
                            )

                pend = None
                for jj in range(NS // 2):
                    j0, j1 = 2 * jj, 2 * jj + 1
                    sc0 = psc.tile([128, S], f32, tag="sc")
                    sc1 = psc.tile([128, S], f32, tag="sc")
                    for c in range(S // QS):
                        cs = slice(c * QS, (c + 1) * QS)
                        nc.tensor.matmul(
                            sc0[:, cs],
                            kT[r0, t, j0 * 128 : (j0 + 1) * 128],
                            qT[r0, t, cs],
                            start=True,
                            stop=True,
                        )
                        nc.tensor.matmul(
                            sc1[:, cs],
                            kT2[r1, t, j1 * 128 : (j1 + 1) * 128],
                            qT2[r1, t, cs],
                            start=True,
                            stop=True,
                        )
                    ets = []
                    for sc in (sc0, sc1):
                        et = exps.tile([128, S], bf16, tag="exp")
                        nc.scalar.activation(et, sc, Exp)
                        ets.append(et)
                    filler(jj)
                    if jj == 0 and prev_tail is not None:
                        prev_tail()
                    if pend is not None:
                        do_ctx(pend)
                    pend = ((j0, ets[0]), (j1, ets[1]))

                cs_sb = fin.tile(
                    [DH + 1, S], bf16, tag="ctx_sb", name=f"cs{h}", bufs=6
                )

                def tail():
                    do_ctx(pend)
                    nc.vector.tensor_copy(cs_sb, ctx_ps)
                    hh = 32 * (h % 4)
                    nc.vector.tensor_copy(
                        dens[hh : hh + 1, :], cs_sb[DH : DH + 1, :]
                    )

                return cs_sb, tail

            def act_recip(lo, hi):
                """1/d = exp(-ln d) on the ACT engine for dens rows [lo, hi).
                DVE's iterative-divide reciprocal (~6.5 cyc/elem) would block
                its strict-FIFO queue and stall PSUM evacuations; two ACT ops
                cost ~2.2us of ACT slack (same table set as exp)."""
                nc.scalar.activation(densf[lo:hi, :], dens[lo:hi, :], Ln)
                nc.scalar.activation(densr[lo:hi, :], densf[lo:hi, :], Exp, scale=-1.0)

            def fin_head(h, cs_sb):
                """PE broadcast of 1/denominator + normalize + store (paced
                ~1 head after the reciprocal so nothing stalls the PE queue)."""
                hh = 32 * (h % 4)
                for c in range(S // QS):
                    cs = slice(c * QS, (c + 1) * QS)
                    bc = pp.tile([DH, QS], f32, tag="proj", name=f"bc{h}_{c}")
                    nc.tensor.matmul(
                        bc,
                        ones97[hh : hh + 1, :],
                        densr[hh : hh + 1, cs],
                        start=True,
                        stop=True,
                        tile_position=(hh, 0),
                    )
                    nc.vector.tensor_tensor(
                        out=cs_sb[0:DH, cs], in0=cs_sb[0:DH, cs], in1=bc, op=mult
                    )
                nc.sync.dma_start(out=out_d[h], in_=cs_sb[0:DH, :])

            # ---- emission schedule ----
            for c in range(2):
                qk_chunk("q", hsT, qT, bqT, 0, c)
            for c in range(2):
                qk_chunk("k", hsTk, kT, bkT, 0, c)
            qk_swap(qT, qT2, 0)
            qk_swap(kT, kT2, 0)

            fillers = {h: {} for h in range(H)}
            fillers[0] = {
                jj: [lambda a=2 * jj, b=2 * jj + 1: (proj_v(a), proj_v(b))]
                for jj in range(4)
            }
            fillers[1] = {
                0: [lambda: qk_chunk("q", hsT, qT, bqT, 1, 0)],
                1: [lambda: qk_chunk("q", hsT, qT, bqT, 1, 1)],
                2: [
                    lambda: qk_chunk("k", hsTk, kT, bkT, 1, 0),
                    lambda: qk_swap(qT, qT2, 1),
                ],
                3: [
                    lambda: qk_chunk("k", hsTk, kT, bkT, 1, 1),
                    lambda: qk_swap(kT, kT2, 1),
                ],
            }
            for h in range(2, 10):
                t = h // 2 + 1
                if h % 2 == 0:
                    w, src, dest, bias, dest2 = "q", hsT, qT, bqT, qT2
                else:
                    w, src, dest, bias, dest2 = "k", hsTk, kT, bkT, kT2
                fillers[h] = {
                    0: [
                        lambda w=w, src=src, dest=dest, bias=bias, t=t: qk_chunk(
                            w, src, dest, bias, t, 0
                        )
                    ],
                    2: [
                        lambda w=w, src=src, dest=dest, bias=bias, t=t: qk_chunk(
                            w, src, dest, bias, t, 1
                        )
                    ],
                    3: [lambda dest=dest, dest2=dest2, t=t: qk_swap(dest, dest2, t)],
                }

            # normalize pacing: a group's exp(-ln d) lands two slots after the
            # last dens row is available; finishers ~1 head later still.
            done = {}
            sched = {
                4: {2: [lambda: act_recip(0, 97)]},
                5: {0: [lambda: fin_head(0, done[0])], 2: [lambda: fin_head(1, done[1])]},
                6: {0: [lambda: fin_head(2, done[2])], 2: [lambda: fin_head(3, done[3])]},
                8: {2: [lambda: act_recip(0, 97)]},
                9: {0: [lambda: fin_head(4, done[4])], 2: [lambda: fin_head(5, done[5])]},
                10: {
                    0: [lambda: fin_head(6, done[6])],
                    2: [lambda: act_recip(0, 33), lambda: fin_head(7, done[7])],
                },
                11: {
                    0: [lambda: fin_head(8, done[8])],
                    1: [lambda: act_recip(64, 65)],
                    2: [lambda: fin_head(9, done[9])],
                    3: [lambda: fin_head(10, done[10])],
                },
            }
            prev_tail = None
            for h in range(H):
                hf = fillers[h]
                extra = sched.get(h, {})

                def fill(jj, hf=hf, extra=extra):
                    for fn in hf.get(jj, []):
                        fn()
                    for fn in extra.get(jj, []):
                        fn()

                done[h], prev_tail = attn_head(h, fill, prev_tail)
            prev_tail()
            act_recip(96, 97)
            fin_head(11, done[11])

    _install_multiwait_split(nc)
    return nc


def _install_multiwait_split(nc):
    """This walrus build accepts at most one sync wait per instruction
    (Drain/CTRL and Matmult/LDWEIGHTS structs at least). Tile attaches
    several. Split extras onto single-wait NoOps inserted just before the
    instruction, at JSON-serialization time so every compile path sees it."""
    import types

    import orjson
    from concourse import mybir

    def to_json_bytes(self):
        m = orjson.loads(mybir.module_to_json_bytes(self.m))
        n = 0
        for fn in m.get("functions", []):
            for bb in fn.get("blocks", []):
                insts = bb.get("instructions", [])
                out = []
                for inst in insts:
                    si = inst.get("sync_info")
                    waits = (si or {}).get("on_wait") or []
                    if len(waits) > 1:
                        for w in waits[:-1]:
                            n += 1
                            out.append(
                                {
                                    "debug": inst.get("debug", 0),
                                    "engine": inst["engine"],
                                    "ins": [],
                                    "name": f"I-mws{n}",
                                    "opcode": "NoOp",
                                    "outs": [],
                                    "sync_info": {"on_update": [], "on_wait": [w]},
                                    "text_hint": "multiwait_split",
                                }
                            )
                        si["on_wait"] = [waits[-1]]
                    out.append(inst)
                bb["instructions"] = out
        return orjson.dumps(m)

    nc.to_json_bytes = types.MethodType(to_json_bytes, nc)


def _get_built():
    global _built
    if _built is None:
        _apply_workarounds()
        _built = _build()
    return _built


def _prep_in_maps(inputs):
    hs = np.asarray(inputs["hidden_states"], np.float32)
    mask = np.asarray(inputs["attention_mask"], np.float32)
    click = np.asarray(inputs["click_times"], np.float32)
    Wq = np.asarray(inputs["Wq"], np.float32)
    bq = np.asarray(inputs["bq"], np.float32)
    Wk = np.asarray(inputs["Wk"], np.float32)
    bk = np.asarray(inputs["bk"], np.float32)
    Wv = np.asarray(inputs["Wv"], np.float32)
    bv = np.asarray(inputs["bv"], np.float32)

    import ml_dtypes

    bf16 = ml_dtypes.bfloat16
    scale = 1.0 / np.sqrt(np.float32(DH))
    shared = {
        "wqT": np.ascontiguousarray(Wq.T).astype(bf16),
        "wkT": np.ascontiguousarray(Wk.T * scale).astype(bf16),
        "wvT": np.ascontiguousarray(Wv.T).astype(bf16),
        "bqT": np.ascontiguousarray(bq.reshape(NT, 128).T),
        "bkT": np.ascontiguousarray((bk * scale).reshape(NT, 128).T),
        "bvB": np.ascontiguousarray(np.broadcast_to(bv, (128, D))),
        "ones97": np.ones((97, DH), np.float32),
        "vones": np.ones((128, NS, H), bf16),
        "perm": np.eye(128, dtype=np.float32)[
            :, list(range(64, 128)) + list(range(64))
        ].astype(bf16),
    }
    in_maps = []
    for b in range(B):
        m = dict(shared)
        hsTb = np.ascontiguousarray(hs[b].T)
        m["hsT"] = hsTb.astype(bf16)
        m["hsTk"] = (hsTb * click[b][None, :]).astype(bf16)
        m["maskT"] = np.ascontiguousarray(
            np.exp(mask[b, 0, 0].astype(np.float64))
            .astype(np.float32)
            .reshape(NS, 128)
            .T
        )
        in_maps.append(m)
    return in_maps


def run(inputs, trace=False, tmpdir=None):
    """Run on the 8 cores; returns (output [B,S,D], BassKernelResults)."""
    from concourse.bass_utils import run_bass_kernel_spmd

    nc = _get_built()
    in_maps = _prep_in_maps(inputs)
    res = run_bass_kernel_spmd(
        nc, in_maps, list(range(B)), trace=trace, tmpdir=tmpdir
    )
    out = np.empty((B, S, D), np.float32)
    for b in range(B):
        ctxT = np.asarray(res.results[b]["out"], dtype=np.float32)  # [H, DH, S]
        out[b] = ctxT.transpose(2, 0, 1).reshape(S, D)
    return out, res


def kernel(**inputs) -> np.ndarray:
    out, _ = run(inputs)
    return out


# revision 25
# speedup vs baseline: 1.2278x; 1.0174x over previous
"""BERT self-attention (B=8, S=1024, D=768, H=12) on 8 TRN2 NeuronCores.

Sharding: batch across the 8 cores (one batch element per core).

Per-core dataflow (all matmuls bf16 on the tensor engine, fp32 PSUM):
  - host pre-transposes hs[b] -> hsT [D, S] and weights -> W.T; 1/sqrt(dh)
    folds into Wk/bk; the click gate folds into a second moving operand
    hsTk = hsT * click[s] used only by the k-projection; exp(mask) folds
    into the v rows (and their ones column), all host-side or free.
  - qT/kT[dout, s] = W.T-tiles (stationary) x hsT/hsTk (moving).
    qT2/kT2 are partition-half-rotated copies made with a 0/1 permutation
    matmul (exact; sum over partitions reindexes), so consecutive-j score
    matmuls land in distinct PE row groups and run concurrently.
  - v[s, dout] head-major [s, (h, 65)] with a ones column (denominator).
  - per head, per j-pair jj: scoresT for j0 (rows r0 of qT/kT) and j1
    (rows r1 of qT2/kT2) packed concurrently; one plain Exp per j
    (PSUM->SBUF bf16); ctx matmuls are software-pipelined ONE jj-slot
    behind (and across heads) so the in-order PE queue never blocks on
    an exp that hasn't finished; ctxT[65, qs] accumulates over j.
  - normalize: 1/denominator = exp(-ln d) on the ACT engine (same table
    set as exp; DVE's iterative reciprocal would block its FIFO ~6.6us),
    batched 4 heads into partition rows; PE partition-broadcast (ones
    stationary at the head's row), multiply, DMA out bf16.
  - host transposes back and upcasts on unshard.
"""

import sys

sys.path.insert(0, "/opt/trn_rl_repo")

import numpy as np

B, S, D, H = 8, 1024, 768, 12
DH = D // H  # 64
NT = D // 128  # 6 dout/din tiles
NS = S // 128  # 8 s tiles
QS = 512  # qs chunk (PSUM bank / fp32 moving max)
NWARM = 22  # PE warmup matmuls ([128,512] each) to cover startup DMA

_built = None


def _apply_workarounds():
    """Container fixes: (1) walrus here accepts at most one sync wait on the
    Tile tail Drain -> split extra waits onto SP nops; (2) antenv.axon_hooks
    is missing from the image (needed only for trace=True profiling)."""
    import os

    import concourse.tile as tile
    from concourse.vector_clock import ScopedClock

    if getattr(tile.TileContext, "_drain_split_patched", False):
        return

    def _drain_and_barrier(self, tick_clock, wait_clock):
        drain_inst = self.nc.sync.drain()
        wait_clock.add_sem_waits(
            drain_inst.ins, ScopedClock({None: tick_clock.global_clock})
        )
        si = drain_inst.ins.sync_info
        if si is not None and len(si.on_wait) > 1:
            waits = list(si.on_wait)
            si.on_wait = waits[:1]
            for w in waits[1:]:
                nop = self.nc.sync.nop(nofuse=True, hint="drain_wait_split")
                nsi = nop.ins.sync_info
                if nsi is None:
                    import bass_rust

                    nop.ins.sync_info = bass_rust.SyncInfo(on_update=[], on_wait=[w])
                else:
                    nsi.on_wait = [w]

        self.nc.all_engine_barrier()
        assert self.sems is not None
        popped = self.nc._tile_sem_poison_stack.pop()
        assert popped is self._sem_poison
        self.nc.clear_and_free_semaphores(list(self.sems.allocated().values()))
        self.nc.all_engine_barrier()

    tile.TileContext._drain_and_barrier = _drain_and_barrier
    tile.TileContext._drain_split_patched = True

    hooks_src = (
        "_axon_ntff_profile_hook = None\n\n\n"
        "def set_axon_ntff_profile_hook(hook):\n"
        "    global _axon_ntff_profile_hook\n"
        "    _axon_ntff_profile_hook = hook\n\n\n"
        "def get_axon_ntff_profile_hook():\n"
        "    return _axon_ntff_profile_hook\n"
    )
    for d in ("/root/.axon_site/_ro/trn_rl_repo/antenv", "/opt/trn_rl_repo/antenv"):
        path = os.path.join(d, "axon_hooks.py")
        try:
            if os.path.isdir(d) and not os.path.exists(path):
                with open(path, "w") as f:
                    f.write(hooks_src)
        except OSError:
            pass


def _build():
    import concourse.bass as bass
    import concourse.tile as tile
    from concourse import mybir

    f32 = mybir.dt.float32
    f32r = mybir.dt.float32r
    bf16 = mybir.dt.bfloat16
    Exp = mybir.ActivationFunctionType.Exp
    Ln = mybir.ActivationFunctionType.Ln
    mult = mybir.AluOpType.mult

    nc = bass.Bass()
    hsT_d = nc.dram_tensor("hsT", [D, S], bf16, kind="ExternalInput")
    hsTk_d = nc.dram_tensor("hsTk", [D, S], bf16, kind="ExternalInput")
    wT_d = {
        w: nc.dram_tensor(f"w{w}T", [D, D], bf16, kind="ExternalInput")
        for w in ("q", "k", "v")
    }
    bqT_d = nc.dram_tensor("bqT", [128, NT], f32, kind="ExternalInput")
    bkT_d = nc.dram_tensor("bkT", [128, NT], f32, kind="ExternalInput")
    bvB_d = nc.dram_tensor("bvB", [128, D], f32, kind="ExternalInput")
    maskT_d = nc.dram_tensor("maskT", [128, NS], f32, kind="ExternalInput")
    ones97_d = nc.dram_tensor("ones97", [97, DH], f32r, kind="ExternalInput")
    vones_d = nc.dram_tensor("vones", [128, NS, H], bf16, kind="ExternalInput")
    perm_d = nc.dram_tensor("perm", [128, 128], bf16, kind="ExternalInput")
    out_d = nc.dram_tensor("out", [H, DH, S], bf16, kind="ExternalOutput")

    with tile.TileContext(nc) as tc:
        from contextlib import ExitStack

        with ExitStack() as ctx:
            consts = ctx.enter_context(tc.tile_pool(name="consts", bufs=1))
            big = ctx.enter_context(tc.tile_pool(name="big", bufs=1))
            exps = ctx.enter_context(tc.tile_pool(name="exps", bufs=5))
            fin = ctx.enter_context(tc.tile_pool(name="fin", bufs=2))
            # PSUM: scores 2x[128,1024]f32 (4 banks) + ctx [65,1024] (2) +
            # proj/bc 2x[128,512] (2) = 8 banks exactly.
            psc = ctx.enter_context(tc.tile_pool(name="psc", bufs=2, space="PSUM"))
            pcx = ctx.enter_context(tc.tile_pool(name="pcx", bufs=1, space="PSUM"))
            pp = ctx.enter_context(tc.tile_pool(name="pp", bufs=2, space="PSUM"))

            # ---- big inputs first (their transfers gate the first matmuls;
            # consts are consumed later) ----
            hsT = big.tile([128, NT, S], bf16)
            hsTk = big.tile([128, NT, S], bf16, tag="hsTk")
            wT = {}
            for w in ("q", "k", "v"):
                wT[w] = big.tile([128, NT, D], bf16, tag=f"w{w}", name=f"w{w}sb")
            # spread the 5 big loads across engine DMA queues so the
            # transfers run in parallel (one queue would serialize ~18us)
            nc.sync.dma_start(out=hsT, in_=hsT_d.rearrange("(t p) s -> p t s", p=128))
            nc.scalar.dma_start(
                out=wT["q"], in_=wT_d["q"].rearrange("(t p) d -> p t d", p=128)
            )
            nc.gpsimd.dma_start(
                out=hsTk, in_=hsTk_d.rearrange("(t p) s -> p t s", p=128)
            )
            nc.sync.dma_start(
                out=wT["k"], in_=wT_d["k"].rearrange("(t p) d -> p t d", p=128)
            )
            nc.scalar.dma_start(
                out=wT["v"], in_=wT_d["v"].rearrange("(t p) d -> p t d", p=128)
            )

            # ---- constants ----
            bqT = consts.tile([128, NT], f32)
            nc.sync.dma_start(out=bqT, in_=bqT_d[:])
            bkT = consts.tile([128, NT], f32)
            nc.sync.dma_start(out=bkT, in_=bkT_d[:])
            bvB = consts.tile([128, D], f32)
            nc.sync.dma_start(out=bvB, in_=bvB_d[:])
            maskT = consts.tile([128, NS], f32)
            nc.sync.dma_start(out=maskT, in_=maskT_d[:])
            ones97 = consts.tile([97, DH], f32r)
            nc.sync.dma_start(out=ones97, in_=ones97_d[:])
            perm = consts.tile([128, 128], bf16)
            nc.sync.dma_start(out=perm, in_=perm_d[:])

            warm = consts.tile([128, QS], bf16, name="warm")
            nc.vector.memset(warm, 0.0)
            # preload the exp/ln activation table set during the DMA phase
            pre = consts.tile([128, 1], f32, name="pre")
            nc.scalar.activation(pre, warm[:, 0:1], Exp)
            for wi in range(NWARM):
                wp = pp.tile([128, QS], f32, tag="proj", name=f"warm{wi}")
                nc.tensor.matmul(wp, warm[:, 0:128], warm, start=True, stop=True)

            qT = big.tile([128, NT, S], bf16, tag="qT")
            kT = big.tile([128, NT, S], bf16, tag="kT")
            qT2 = big.tile([128, NT, S], bf16, tag="qT2")
            kT2 = big.tile([128, NT, S], bf16, tag="kT2")
            # v_aug: [s_partition, s_tile, head-major (h, dh | ones)]
            v = big.tile([128, NS, H * (DH + 1)], bf16, tag="v")
            dens = big.tile([97, S], f32, tag="dens")
            nc.vector.memset(dens, 1.0)
            densf = big.tile([97, S], f32, tag="densf")
            densr = big.tile([97, S], f32r, tag="densr")

            def qk_chunk(w, src, dest, bias, t, c):
                cs = slice(c * QS, (c + 1) * QS)
                ps = pp.tile([128, QS], f32, tag="proj")
                for k in range(NT):
                    nc.tensor.matmul(
                        ps,
                        wT[w][:, k, t * 128 : (t + 1) * 128],
                        src[:, k, cs],
                        start=(k == 0),
                        stop=(k == NT - 1),
                    )
                nc.vector.tensor_scalar_add(dest[:, t, cs], ps, bias[:, t : t + 1])

            def qk_swap(dest, dest2, t):
                # partition-halves rotation via a 0/1 permutation matmul
                # (exact in bf16): head data relocates to the other PE row
                # group so consecutive-j score matmuls run concurrently.
                for c in range(S // QS):
                    cs = slice(c * QS, (c + 1) * QS)
                    ps = pp.tile([128, QS], f32, tag="proj")
                    nc.tensor.matmul(ps, perm, dest[:, t, cs], start=True, stop=True)
                    nc.vector.tensor_copy(dest2[:, t, cs], ps)

            def proj_v(si):
                """v rows for s-tile si, head-major with ones col, scaled by
                exp(mask) (folds the additive mask through the softmax)."""
                vsi = v[:, si, :].rearrange("p (h e) -> p h e", e=DH + 1)
                for c0, cn in ((0, 512), (512, 256)):
                    h0, nh = c0 // DH, cn // DH
                    ps = pp.tile([128, cn], f32, tag="proj")
                    for k in range(NT):
                        nc.tensor.matmul(
                            ps,
                            hsT[:, k, si * 128 : (si + 1) * 128],
                            wT["v"][:, k, c0 : c0 + cn],
                            start=(k == 0),
                            stop=(k == NT - 1),
                        )
                    nc.vector.tensor_tensor(
                        out=vsi[:, h0 : h0 + nh, 0:DH],
                        in0=ps.rearrange("p (h e) -> p h e", e=DH),
                        in1=bvB[:, c0 : c0 + cn].rearrange("p (h e) -> p h e", e=DH),
                        op=mybir.AluOpType.add,
                    )
                nc.sync.dma_start(out=vsi[:, :, DH : DH + 1], in_=vones_d[:, si, :])
                nc.vector.tensor_scalar_mul(
                    v[:, si, :], v[:, si, :], maskT[:, si : si + 1]
                )

            def attn_head(h, filler, prev_tail):
                """One head. ctx matmuls run one jj-slot behind the scores
                (software pipeline) so the in-order PE queue never parks on
                an exp still in flight; the previous head's last ctx + PSUM
                evacuation are emitted inside this head's slot 0."""
                t = h // 2
                lo, hi = slice(0, 64), slice(64, 128)
                r0 = hi if h % 2 else lo  # head's rows in qT/kT
                r1 = lo if h % 2 else hi  # head's rows in qT2/kT2
                ctx_ps = pcx.tile([DH + 1, S], f32, tag="ctx", name=f"ctx{h}")

                def do_ctx(pend):
                    for j, et in pend:
                        va = v[:, j, :].rearrange("p (h e) -> p h e", e=DH + 1)[
                            :, h, :
                        ]
                        for c in range(S // QS):
                            cs = slice(c * QS, (c + 1) * QS)
                            nc.tensor.matmul(
                                ctx_ps[:, cs],
                                va,
                                et[:, cs],
                                start=(j == 0),
                                stop=(j == NS - 1),
                            )

                pend = None
                for jj in range(NS // 2):
                    j0, j1 = 2 * jj, 2 * jj + 1
                    sc0 = psc.tile([128, S], f32, tag="sc")
                    sc1 = psc.tile([128, S], f32, tag="sc")
                    for c in range(S // QS):
                        cs = slice(c * QS, (c + 1) * QS)
                        nc.tensor.matmul(
                            sc0[:, cs],
                            kT[r0, t, j0 * 128 : (j0 + 1) * 128],
                            qT[r0, t, cs],
                            start=True,
                            stop=True,
                        )
                        nc.tensor.matmul(
                            sc1[:, cs],
                            kT2[r1, t, j1 * 128 : (j1 + 1) * 128],
                            qT2[r1, t, cs],
                            start=True,
                            stop=True,
                        )
                    ets = []
                    for sc in (sc0, sc1):
                        et = exps.tile([128, S], bf16, tag="exp")
                        nc.scalar.activation(et, sc, Exp)
                        ets.append(et)
                    filler(jj)
                    if jj == 0 and prev_tail is not None:
                        prev_tail()
                    if pend is not None:
                        do_ctx(pend)
                    pend = ((j0, ets[0]), (j1, ets[1]))

                cs_sb = fin.tile(
                    [DH + 1, S], bf16, tag="ctx_sb", name=f"cs{h}", bufs=6
                )

                def tail():
                    do_ctx(pend)
                    nc.vector.tensor_copy(cs_sb, ctx_ps)
                    hh = 32 * (h % 4)
                    nc.vector.tensor_copy(
                        dens[hh : hh + 1, :], cs_sb[DH : DH + 1, :]
                    )

                return cs_sb, tail

            def act_recip(lo, hi):
                """1/d = exp(-ln d) on the ACT engine for dens rows [lo, hi).
                DVE's iterative-divide reciprocal (~6.5 cyc/elem) would block
                its strict-FIFO queue and stall PSUM evacuations; two ACT ops
                cost ~2.2us of ACT slack (same table set as exp)."""
                nc.scalar.activation(densf[lo:hi, :], dens[lo:hi, :], Ln)
                nc.scalar.activation(densr[lo:hi, :], densf[lo:hi, :], Exp, scale=-1.0)

            def fin_head(h, cs_sb):
                """PE broadcast of 1/denominator + normalize + store (paced
                ~1 head after the reciprocal so nothing stalls the PE queue)."""
                hh = 32 * (h % 4)
                for c in range(S // QS):
                    cs = slice(c * QS, (c + 1) * QS)
                    bc = pp.tile([DH, QS], f32, tag="proj", name=f"bc{h}_{c}")
                    nc.tensor.matmul(
                        bc,
                        ones97[hh : hh + 1, :],
                        densr[hh : hh + 1, cs],
                        start=True,
                        stop=True,
                        tile_position=(hh, 0),
                    )
                    nc.vector.tensor_tensor(
                        out=cs_sb[0:DH, cs], in0=cs_sb[0:DH, cs], in1=bc, op=mult
                    )
                nc.sync.dma_start(out=out_d[h], in_=cs_sb[0:DH, :])

            # ---- emission schedule ----
            for c in range(2):
                qk_chunk("q", hsT, qT, bqT, 0, c)
            for c in range(2):
                qk_chunk("k", hsTk, kT, bkT, 0, c)
            qk_swap(qT, qT2, 0)
            qk_swap(kT, kT2, 0)

            fillers = {h: {} for h in range(H)}
            # v(si) pair feeds ctx(0, si//2) which runs one slot later
            # (pipelined), so the pairs spread across head 0 + head 1 slot 0.
            fillers[0] = {
                0: [lambda: proj_v(0), lambda: proj_v(1)],
                1: [lambda: proj_v(2)],
                2: [lambda: proj_v(3), lambda: proj_v(4)],
                3: [lambda: proj_v(5)],
            }
            fillers[1] = {
                0: [lambda: proj_v(6), lambda: proj_v(7)],
                1: [lambda: qk_chunk("q", hsT, qT, bqT, 1, 0)],
                2: [
                    lambda: qk_chunk("q", hsT, qT, bqT, 1, 1),
                    lambda: qk_chunk("k", hsTk, kT, bkT, 1, 0),
                    lambda: qk_swap(qT, qT2, 1),
                ],
                3: [
                    lambda: qk_chunk("k", hsTk, kT, bkT, 1, 1),
                    lambda: qk_swap(kT, kT2, 1),
                ],
            }
            for h in range(2, 10):
                t = h // 2 + 1
                if h % 2 == 0:
                    w, src, dest, bias, dest2 = "q", hsT, qT, bqT, qT2
                else:
                    w, src, dest, bias, dest2 = "k", hsTk, kT, bkT, kT2
                fillers[h] = {
                    0: [
                        lambda w=w, src=src, dest=dest, bias=bias, t=t: qk_chunk(
                            w, src, dest, bias, t, 0
                        )
                    ],
                    2: [
                        lambda w=w, src=src, dest=dest, bias=bias, t=t: qk_chunk(
                            w, src, dest, bias, t, 1
                        )
                    ],
                    3: [lambda dest=dest, dest2=dest2, t=t: qk_swap(dest, dest2, t)],
                }

            # normalize pacing: a group's exp(-ln d) lands two slots after the
            # last dens row is available; finishers ~1 head later still.
            done = {}
            sched = {
                4: {2: [lambda: act_recip(0, 97)]},
                5: {0: [lambda: fin_head(0, done[0])], 2: [lambda: fin_head(1, done[1])]},
                6: {0: [lambda: fin_head(2, done[2])], 2: [lambda: fin_head(3, done[3])]},
                8: {2: [lambda: act_recip(0, 97)]},
                9: {0: [lambda: fin_head(4, done[4])], 2: [lambda: fin_head(5, done[5])]},
                10: {
                    0: [lambda: fin_head(6, done[6])],
                    2: [lambda: act_recip(0, 33), lambda: fin_head(7, done[7])],
                },
                11: {
                    0: [lambda: fin_head(8, done[8])],
                    1: [lambda: act_recip(64, 65)],
                    2: [lambda: fin_head(9, done[9])],
                    3: [lambda: fin_head(10, done[10])],
                },
            }
            prev_tail = None
            for h in range(H):
                hf = fillers[h]
                extra = sched.get(h, {})

                def fill(jj, hf=hf, extra=extra):
                    for fn in hf.get(jj, []):
                        fn()
                    for fn in extra.get(jj, []):
                        fn()

                done[h], prev_tail = attn_head(h, fill, prev_tail)
            prev_tail()
            act_recip(96, 97)
            fin_head(11, done[11])

    _install_multiwait_split(nc)
    return nc


def _install_multiwait_split(nc):
    """This walrus build accepts at most one sync wait per instruction
    (Drain/CTRL and Matmult/LDWEIGHTS structs at least). Tile attaches
    several. Split extras onto single-wait NoOps inserted just before the
    instruction, at JSON-serialization time so every compile path sees it."""
    import types

    import orjson
    from concourse import mybir

    def to_json_bytes(self):
        m = orjson.loads(mybir.module_to_json_bytes(self.m))
        n = 0
        for fn in m.get("functions", []):
            for bb in fn.get("blocks", []):
                insts = bb.get("instructions", [])
                out = []
                for inst in insts:
                    si = inst.get("sync_info")
                    waits = (si or {}).get("on_wait") or []
                    if len(waits) > 1:
                        for w in waits[:-1]:
                            n += 1
                            out.append(
                                {
                                    "debug": inst.get("debug", 0),
                                    "engine": inst["engine"],
                                    "ins": [],
                                    "name": f"I-mws{n}",
                                    "opcode": "NoOp",
                                    "outs": [],
                                    "sync_info": {"on_update": [], "on_wait": [w]},
                                    "text_hint": "multiwait_split",
                                }
                            )
                        si["on_wait"] = [waits[-1]]
                    out.append(inst)
                bb["instructions"] = out
        return orjson.dumps(m)

    nc.to_json_bytes = types.MethodType(to_json_bytes, nc)


def _get_built():
    global _built
    if _built is None:
        _apply_workarounds()
        _built = _build()
    return _built


def _prep_in_maps(inputs):
    hs = np.asarray(inputs["hidden_states"], np.float32)
    mask = np.asarray(inputs["attention_mask"], np.float32)
    click = np.asarray(inputs["click_times"], np.float32)
    Wq = np.asarray(inputs["Wq"], np.float32)
    bq = np.asarray(inputs["bq"], np.float32)
    Wk = np.asarray(inputs["Wk"], np.float32)
    bk = np.asarray(inputs["bk"], np.float32)
    Wv = np.asarray(inputs["Wv"], np.float32)
    bv = np.asarray(inputs["bv"], np.float32)

    import ml_dtypes

    bf16 = ml_dtypes.bfloat16
    scale = 1.0 / np.sqrt(np.float32(DH))
    shared = {
        "wqT": np.ascontiguousarray(Wq.T).astype(bf16),
        "wkT": np.ascontiguousarray(Wk.T * scale).astype(bf16),
        "wvT": np.ascontiguousarray(Wv.T).astype(bf16),
        "bqT": np.ascontiguousarray(bq.reshape(NT, 128).T),
        "bkT": np.ascontiguousarray((bk * scale).reshape(NT, 128).T),
        "bvB": np.ascontiguousarray(np.broadcast_to(bv, (128, D))),
        "ones97": np.ones((97, DH), np.float32),
        "vones": np.ones((128, NS, H), bf16),
        "perm": np.eye(128, dtype=np.float32)[
            :, list(range(64, 128)) + list(range(64))
        ].astype(bf16),
    }
    in_maps = []
    for b in range(B):
        m = dict(shared)
        hsTb = np.ascontiguousarray(hs[b].T)
        m["hsT"] = hsTb.astype(bf16)
        m["hsTk"] = (hsTb * click[b][None, :]).astype(bf16)
        m["maskT"] = np.ascontiguousarray(
            np.exp(mask[b, 0, 0].astype(np.float64))
            .astype(np.float32)
            .reshape(NS, 128)
            .T
        )
        in_maps.append(m)
    return in_maps


def run(inputs, trace=False, tmpdir=None):
    """Run on the 8 cores; returns (output [B,S,D], BassKernelResults)."""
    from concourse.bass_utils import run_bass_kernel_spmd

    nc = _get_built()
    in_maps = _prep_in_maps(inputs)
    res = run_bass_kernel_spmd(
        nc, in_maps, list(range(B)), trace=trace, tmpdir=tmpdir
    )
    out = np.empty((B, S, D), np.float32)
    for b in range(B):
        ctxT = np.asarray(res.results[b]["out"], dtype=np.float32)  # [H, DH, S]
        out[b] = ctxT.transpose(2, 0, 1).reshape(S, D)
    return out, res


def kernel(**inputs) -> np.ndarray:
    out, _ = run(inputs)
    return out


# revision 28
# speedup vs baseline: 1.2402x; 1.0101x over previous
"""BERT self-attention (B=8, S=1024, D=768, H=12) on 8 TRN2 NeuronCores.

Sharding: batch across the 8 cores (one batch element per core).

Per-core dataflow (all matmuls bf16 on the tensor engine, fp32 PSUM):
  - host pre-transposes hs[b] -> hsT [D, S] and weights -> W.T; 1/sqrt(dh)
    folds into Wk/bk; the click gate folds into a second moving operand
    hsTk = hsT * click[s] used only by the k-projection; exp(mask) folds
    into the v rows (and their ones column), all host-side or free.
  - qT/kT[dout, s] = W.T-tiles (stationary) x hsT/hsTk (moving).
    qT2/kT2 are partition-half-rotated copies made with a 0/1 permutation
    matmul (exact; sum over partitions reindexes), so consecutive-j score
    matmuls land in distinct PE row groups and run concurrently.
  - v[s, dout] head-major [s, (h, 65)] with a ones column (denominator).
  - per head, per j-pair jj: scoresT for j0 (rows r0 of qT/kT) and j1
    (rows r1 of qT2/kT2) packed concurrently; one plain Exp per j
    (PSUM->SBUF bf16); ctx matmuls are software-pipelined ONE jj-slot
    behind (and across heads) so the in-order PE queue never blocks on
    an exp that hasn't finished; ctxT[65, qs] accumulates over j.
  - normalize: 1/denominator = exp(-ln d) on the ACT engine (same table
    set as exp; DVE's iterative reciprocal would block its FIFO ~6.6us),
    batched 4 heads into partition rows; PE partition-broadcast (ones
    stationary at the head's row), multiply, DMA out bf16.
  - host transposes back and upcasts on unshard.
"""

import sys

sys.path.insert(0, "/opt/trn_rl_repo")

import numpy as np

B, S, D, H = 8, 1024, 768, 12
DH = D // H  # 64
NT = D // 128  # 6 dout/din tiles
NS = S // 128  # 8 s tiles
QS = 512  # qs chunk (PSUM bank / fp32 moving max)
NWARM = 22  # PE warmup matmuls ([128,512] each) to cover startup DMA

_built = None


def _apply_workarounds():
    """Container fixes: (1) walrus here accepts at most one sync wait on the
    Tile tail Drain -> split extra waits onto SP nops; (2) antenv.axon_hooks
    is missing from the image (needed only for trace=True profiling)."""
    import os

    import concourse.tile as tile
    from concourse.vector_clock import ScopedClock

    if getattr(tile.TileContext, "_drain_split_patched", False):
        return

    def _drain_and_barrier(self, tick_clock, wait_clock):
        drain_inst = self.nc.sync.drain()
        wait_clock.add_sem_waits(
            drain_inst.ins, ScopedClock({None: tick_clock.global_clock})
        )
        si = drain_inst.ins.sync_info
        if si is not None and len(si.on_wait) > 1:
            waits = list(si.on_wait)
            si.on_wait = waits[:1]
            for w in waits[1:]:
                nop = self.nc.sync.nop(nofuse=True, hint="drain_wait_split")
                nsi = nop.ins.sync_info
                if nsi is None:
                    import bass_rust

                    nop.ins.sync_info = bass_rust.SyncInfo(on_update=[], on_wait=[w])
                else:
                    nsi.on_wait = [w]

        self.nc.all_engine_barrier()
        assert self.sems is not None
        popped = self.nc._tile_sem_poison_stack.pop()
        assert popped is self._sem_poison
        self.nc.clear_and_free_semaphores(list(self.sems.allocated().values()))
        self.nc.all_engine_barrier()

    tile.TileContext._drain_and_barrier = _drain_and_barrier
    tile.TileContext._drain_split_patched = True

    hooks_src = (
        "_axon_ntff_profile_hook = None\n\n\n"
        "def set_axon_ntff_profile_hook(hook):\n"
        "    global _axon_ntff_profile_hook\n"
        "    _axon_ntff_profile_hook = hook\n\n\n"
        "def get_axon_ntff_profile_hook():\n"
        "    return _axon_ntff_profile_hook\n"
    )
    for d in ("/root/.axon_site/_ro/trn_rl_repo/antenv", "/opt/trn_rl_repo/antenv"):
        path = os.path.join(d, "axon_hooks.py")
        try:
            if os.path.isdir(d) and not os.path.exists(path):
                with open(path, "w") as f:
                    f.write(hooks_src)
        except OSError:
            pass


def _build():
    import concourse.bass as bass
    import concourse.tile as tile
    from concourse import mybir

    f32 = mybir.dt.float32
    f32r = mybir.dt.float32r
    bf16 = mybir.dt.bfloat16
    Exp = mybir.ActivationFunctionType.Exp
    Ln = mybir.ActivationFunctionType.Ln
    mult = mybir.AluOpType.mult

    nc = bass.Bass()
    hsT_d = nc.dram_tensor("hsT", [D, S], bf16, kind="ExternalInput")
    hsTk_d = nc.dram_tensor("hsTk", [D, S], bf16, kind="ExternalInput")
    wT_d = {
        w: nc.dram_tensor(f"w{w}T", [D, D], bf16, kind="ExternalInput")
        for w in ("q", "k", "v")
    }
    bqT_d = nc.dram_tensor("bqT", [128, NT], f32, kind="ExternalInput")
    bkT_d = nc.dram_tensor("bkT", [128, NT], f32, kind="ExternalInput")
    bvB_d = nc.dram_tensor("bvB", [128, D], f32, kind="ExternalInput")
    maskT_d = nc.dram_tensor("maskT", [128, NS], f32, kind="ExternalInput")
    ones97_d = nc.dram_tensor("ones97", [97, DH], f32r, kind="ExternalInput")
    vones_d = nc.dram_tensor("vones", [128, NS, H], bf16, kind="ExternalInput")
    perm_d = nc.dram_tensor("perm", [128, 128], bf16, kind="ExternalInput")
    out_d = nc.dram_tensor("out", [H, DH, S], bf16, kind="ExternalOutput")

    with tile.TileContext(nc) as tc:
        from contextlib import ExitStack

        with ExitStack() as ctx:
            consts = ctx.enter_context(tc.tile_pool(name="consts", bufs=1))
            big = ctx.enter_context(tc.tile_pool(name="big", bufs=1))
            exps = ctx.enter_context(tc.tile_pool(name="exps", bufs=5))
            fin = ctx.enter_context(tc.tile_pool(name="fin", bufs=2))
            # PSUM: scores 2x[128,1024]f32 (4 banks) + ctx [65,1024] (2) +
            # proj/bc 2x[128,512] (2) = 8 banks exactly.
            psc = ctx.enter_context(tc.tile_pool(name="psc", bufs=2, space="PSUM"))
            pcx = ctx.enter_context(tc.tile_pool(name="pcx", bufs=1, space="PSUM"))
            pp = ctx.enter_context(tc.tile_pool(name="pp", bufs=2, space="PSUM"))

            # ---- big inputs first (their transfers gate the first matmuls;
            # consts are consumed later) ----
            hsT = big.tile([128, NT, S], bf16)
            hsTk = big.tile([128, NT, S], bf16, tag="hsTk")
            wT = {}
            for w in ("q", "k", "v"):
                wT[w] = big.tile([128, NT, D], bf16, tag=f"w{w}", name=f"w{w}sb")
            # spread the 5 big loads across engine DMA queues so the
            # transfers run in parallel (one queue would serialize ~18us)
            nc.sync.dma_start(out=hsT, in_=hsT_d.rearrange("(t p) s -> p t s", p=128))
            nc.scalar.dma_start(
                out=wT["q"], in_=wT_d["q"].rearrange("(t p) d -> p t d", p=128)
            )
            nc.gpsimd.dma_start(
                out=hsTk, in_=hsTk_d.rearrange("(t p) s -> p t s", p=128)
            )
            nc.sync.dma_start(
                out=wT["k"], in_=wT_d["k"].rearrange("(t p) d -> p t d", p=128)
            )
            nc.scalar.dma_start(
                out=wT["v"], in_=wT_d["v"].rearrange("(t p) d -> p t d", p=128)
            )

            # ---- constants ----
            bqT = consts.tile([128, NT], f32)
            nc.sync.dma_start(out=bqT, in_=bqT_d[:])
            bkT = consts.tile([128, NT], f32)
            nc.sync.dma_start(out=bkT, in_=bkT_d[:])
            bvB = consts.tile([128, D], f32)
            nc.sync.dma_start(out=bvB, in_=bvB_d[:])
            maskT = consts.tile([128, NS], f32)
            nc.sync.dma_start(out=maskT, in_=maskT_d[:])
            ones97 = consts.tile([97, DH], f32r)
            nc.sync.dma_start(out=ones97, in_=ones97_d[:])
            perm = consts.tile([128, 128], bf16)
            nc.sync.dma_start(out=perm, in_=perm_d[:])

            warm = consts.tile([128, QS], bf16, name="warm")
            nc.vector.memset(warm, 0.0)
            # preload the exp/ln activation table set during the DMA phase
            pre = consts.tile([128, 1], f32, name="pre")
            nc.scalar.activation(pre, warm[:, 0:1], Exp)
            for wi in range(NWARM):
                wp = pp.tile([128, QS], f32, tag="proj", name=f"warm{wi}")
                nc.tensor.matmul(wp, warm[:, 0:128], warm, start=True, stop=True)

            qT = big.tile([128, NT, S], bf16, tag="qT")
            kT = big.tile([128, NT, S], bf16, tag="kT")
            qT2 = big.tile([128, NT, S], bf16, tag="qT2")
            kT2 = big.tile([128, NT, S], bf16, tag="kT2")
            # v_aug: [s_partition, s_tile, head-major (h, dh | ones)]
            v = big.tile([128, NS, H * (DH + 1)], bf16, tag="v")
            dens = big.tile([97, S], f32, tag="dens")
            nc.vector.memset(dens, 1.0)
            densf = big.tile([97, S], f32, tag="densf")
            densr = big.tile([97, S], f32r, tag="densr")

            def qk_chunk(w, src, dest, bias, t, c):
                cs = slice(c * QS, (c + 1) * QS)
                ps = pp.tile([128, QS], f32, tag="proj")
                for k in range(NT):
                    nc.tensor.matmul(
                        ps,
                        wT[w][:, k, t * 128 : (t + 1) * 128],
                        src[:, k, cs],
                        start=(k == 0),
                        stop=(k == NT - 1),
                    )
                nc.vector.tensor_scalar_add(dest[:, t, cs], ps, bias[:, t : t + 1])

            def qk_swap(dest, dest2, t):
                # partition-halves rotation via a 0/1 permutation matmul
                # (exact in bf16): head data relocates to the other PE row
                # group so consecutive-j score matmuls run concurrently.
                for c in range(S // QS):
                    cs = slice(c * QS, (c + 1) * QS)
                    ps = pp.tile([128, QS], f32, tag="proj")
                    nc.tensor.matmul(ps, perm, dest[:, t, cs], start=True, stop=True)
                    nc.vector.tensor_copy(dest2[:, t, cs], ps)

            def proj_v(si):
                """v rows for s-tile si, head-major with ones col, scaled by
                exp(mask) (folds the additive mask through the softmax)."""
                vsi = v[:, si, :].rearrange("p (h e) -> p h e", e=DH + 1)
                for c0, cn in ((0, 512), (512, 256)):
                    h0, nh = c0 // DH, cn // DH
                    ps = pp.tile([128, cn], f32, tag="proj")
                    for k in range(NT):
                        nc.tensor.matmul(
                            ps,
                            hsT[:, k, si * 128 : (si + 1) * 128],
                            wT["v"][:, k, c0 : c0 + cn],
                            start=(k == 0),
                            stop=(k == NT - 1),
                        )
                    nc.vector.tensor_tensor(
                        out=vsi[:, h0 : h0 + nh, 0:DH],
                        in0=ps.rearrange("p (h e) -> p h e", e=DH),
                        in1=bvB[:, c0 : c0 + cn].rearrange("p (h e) -> p h e", e=DH),
                        op=mybir.AluOpType.add,
                    )
                nc.sync.dma_start(out=vsi[:, :, DH : DH + 1], in_=vones_d[:, si, :])
                nc.vector.tensor_scalar_mul(
                    v[:, si, :], v[:, si, :], maskT[:, si : si + 1]
                )

            def attn_head(h, filler, prev_tail):
                """One head. ctx matmuls run one jj-slot behind the scores
                (software pipeline) so the in-order PE queue never parks on
                an exp still in flight; the previous head's last ctx + PSUM
                evacuation are emitted inside this head's slot 0."""
                t = h // 2
                lo, hi = slice(0, 64), slice(64, 128)
                r0 = hi if h % 2 else lo  # head's rows in qT/kT
                r1 = lo if h % 2 else hi  # head's rows in qT2/kT2
                ctx_ps = pcx.tile([DH + 1, S], f32, tag="ctx", name=f"ctx{h}")

                def do_ctx(pend):
                    for j, et in pend:
                        va = v[:, j, :].rearrange("p (h e) -> p h e", e=DH + 1)[
                            :, h, :
                        ]
                        for c in range(S // QS):
                            cs = slice(c * QS, (c + 1) * QS)
                            nc.tensor.matmul(
                                ctx_ps[:, cs],
                                va,
                                et[:, cs],
                                start=(j == 0),
                                stop=(j == NS - 1),
                            )

                pend = None
                for jj in range(NS // 2):
                    j0, j1 = 2 * jj, 2 * jj + 1
                    sc0 = psc.tile([128, S], f32, tag="sc")
                    sc1 = psc.tile([128, S], f32, tag="sc")
                    for c in range(S // QS):
                        cs = slice(c * QS, (c + 1) * QS)
                        nc.tensor.matmul(
                            sc0[:, cs],
                            kT[r0, t, j0 * 128 : (j0 + 1) * 128],
                            qT[r0, t, cs],
                            start=True,
                            stop=True,
                        )
                        nc.tensor.matmul(
                            sc1[:, cs],
                            kT2[r1, t, j1 * 128 : (j1 + 1) * 128],
                            qT2[r1, t, cs],
                            start=True,
                            stop=True,
                        )
                    ets = []
                    for sc in (sc0, sc1):
                        et = exps.tile([128, S], bf16, tag="exp")
                        nc.scalar.activation(et, sc, Exp)
                        ets.append(et)
                    filler(jj)
                    if jj == 0 and prev_tail is not None:
                        prev_tail()
                    if pend is not None:
                        do_ctx(pend)
                    pend = ((j0, ets[0]), (j1, ets[1]))

                cs_sb = fin.tile(
                    [DH + 1, S], bf16, tag="ctx_sb", name=f"cs{h}", bufs=6
                )

                def tail():
                    do_ctx(pend)
                    nc.vector.tensor_copy(cs_sb, ctx_ps)
                    hh = 32 * (h % 4)
                    nc.vector.tensor_copy(
                        dens[hh : hh + 1, :], cs_sb[DH : DH + 1, :]
                    )

                return cs_sb, tail

            def act_recip(lo, hi):
                """1/d = exp(-ln d) on the ACT engine for dens rows [lo, hi).
                DVE's iterative-divide reciprocal (~6.5 cyc/elem) would block
                its strict-FIFO queue and stall PSUM evacuations; two ACT ops
                cost ~2.2us of ACT slack (same table set as exp)."""
                nc.scalar.activation(densf[lo:hi, :], dens[lo:hi, :], Ln)
                nc.scalar.activation(densr[lo:hi, :], densf[lo:hi, :], Exp, scale=-1.0)

            def fin_head(h, cs_sb):
                """PE broadcast of 1/denominator + normalize + store (paced
                ~1 head after the reciprocal so nothing stalls the PE queue)."""
                hh = 32 * (h % 4)
                for c in range(S // QS):
                    cs = slice(c * QS, (c + 1) * QS)
                    bc = pp.tile([DH, QS], f32, tag="proj", name=f"bc{h}_{c}")
                    nc.tensor.matmul(
                        bc,
                        ones97[hh : hh + 1, :],
                        densr[hh : hh + 1, cs],
                        start=True,
                        stop=True,
                        tile_position=(hh, 0),
                    )
                    nc.vector.tensor_tensor(
                        out=cs_sb[0:DH, cs], in0=cs_sb[0:DH, cs], in1=bc, op=mult
                    )
                nc.sync.dma_start(out=out_d[h], in_=cs_sb[0:DH, :])

            # ---- emission schedule ----
            for c in range(2):
                qk_chunk("q", hsT, qT, bqT, 0, c)
            for c in range(2):
                qk_chunk("k", hsTk, kT, bkT, 0, c)
            qk_swap(qT, qT2, 0)
            qk_swap(kT, kT2, 0)

            fillers = {h: {} for h in range(H)}
            # v(si) pair feeds ctx(0, si//2) which runs one slot later
            # (pipelined), so the pairs spread across head 0 + head 1 slot 0.
            fillers[0] = {
                0: [lambda: proj_v(0), lambda: proj_v(1)],
                1: [lambda: proj_v(2)],
                2: [lambda: proj_v(3), lambda: proj_v(4)],
                3: [lambda: proj_v(5)],
            }
            fillers[1] = {
                0: [lambda: proj_v(6), lambda: proj_v(7)],
                1: [lambda: qk_chunk("q", hsT, qT, bqT, 1, 0)],
                2: [
                    lambda: qk_chunk("q", hsT, qT, bqT, 1, 1),
                    lambda: qk_chunk("k", hsTk, kT, bkT, 1, 0),
                    lambda: qk_swap(qT, qT2, 1),
                ],
                3: [
                    lambda: qk_chunk("k", hsTk, kT, bkT, 1, 1),
                    lambda: qk_swap(kT, kT2, 1),
                ],
            }
            for h in range(2, 10):
                t = h // 2 + 1
                if h % 2 == 0:
                    w, src, dest, bias, dest2 = "q", hsT, qT, bqT, qT2
                else:
                    w, src, dest, bias, dest2 = "k", hsTk, kT, bkT, kT2
                fillers[h] = {
                    1: [
                        lambda w=w, src=src, dest=dest, bias=bias, t=t: qk_chunk(
                            w, src, dest, bias, t, 0
                        )
                    ],
                    2: [
                        lambda w=w, src=src, dest=dest, bias=bias, t=t: qk_chunk(
                            w, src, dest, bias, t, 1
                        )
                    ],
                    3: [lambda dest=dest, dest2=dest2, t=t: qk_swap(dest, dest2, t)],
                }

            # normalize pacing: a group's exp(-ln d) lands two slots after the
            # last dens row is available; bc finishers >=2 slots later; the
            # last heads' finishers pull forward into heads 10-11's light
            # slots so the kernel tail is just head 11's own chain.
            done = {}
            sched = {
                4: {2: [lambda: act_recip(0, 97)]},
                5: {0: [lambda: fin_head(0, done[0])], 3: [lambda: fin_head(1, done[1])]},
                6: {0: [lambda: fin_head(2, done[2])], 3: [lambda: fin_head(3, done[3])]},
                8: {2: [lambda: act_recip(0, 97)]},
                9: {0: [lambda: fin_head(4, done[4])], 3: [lambda: fin_head(5, done[5])]},
                10: {
                    0: [lambda: fin_head(6, done[6])],
                    2: [lambda: act_recip(0, 33)],
                    3: [lambda: fin_head(7, done[7])],
                },
                11: {
                    0: [lambda: fin_head(8, done[8])],
                    1: [lambda: act_recip(64, 65), lambda: fin_head(9, done[9])],
                    3: [lambda: fin_head(10, done[10])],
                },
            }
            prev_tail = None
            for h in range(H):
                hf = fillers[h]
                extra = sched.get(h, {})

                def fill(jj, hf=hf, extra=extra):
                    for fn in hf.get(jj, []):
                        fn()
                    for fn in extra.get(jj, []):
                        fn()

                done[h], prev_tail = attn_head(h, fill, prev_tail)
            prev_tail()
            act_recip(96, 97)
            fin_head(11, done[11])

    _install_multiwait_split(nc)
    return nc


def _install_multiwait_split(nc):
    """This walrus build accepts at most one sync wait per instruction
    (Drain/CTRL and Matmult/LDWEIGHTS structs at least). Tile attaches
    several. Split extras onto single-wait NoOps inserted just before the
    instruction, at JSON-serialization time so every compile path sees it."""
    import types

    import orjson
    from concourse import mybir

    def to_json_bytes(self):
        m = orjson.loads(mybir.module_to_json_bytes(self.m))
        n = 0
        for fn in m.get("functions", []):
            for bb in fn.get("blocks", []):
                insts = bb.get("instructions", [])
                out = []
                for inst in insts:
                    si = inst.get("sync_info")
                    waits = (si or {}).get("on_wait") or []
                    if len(waits) > 1:
                        for w in waits[:-1]:
                            n += 1
                            out.append(
                                {
                                    "debug": inst.get("debug", 0),
                                    "engine": inst["engine"],
                                    "ins": [],
                                    "name": f"I-mws{n}",
                                    "opcode": "NoOp",
                                    "outs": [],
                                    "sync_info": {"on_update": [], "on_wait": [w]},
                                    "text_hint": "multiwait_split",
                                }
                            )
                        si["on_wait"] = [waits[-1]]
                    out.append(inst)
                bb["instructions"] = out
        return orjson.dumps(m)

    nc.to_json_bytes = types.MethodType(to_json_bytes, nc)


def _get_built():
    global _built
    if _built is None:
        _apply_workarounds()
        _built = _build()
    return _built


def _prep_in_maps(inputs):
    hs = np.asarray(inputs["hidden_states"], np.float32)
    mask = np.asarray(inputs["attention_mask"], np.float32)
    click = np.asarray(inputs["click_times"], np.float32)
    Wq = np.asarray(inputs["Wq"], np.float32)
    bq = np.asarray(inputs["bq"], np.float32)
    Wk = np.asarray(inputs["Wk"], np.float32)
    bk = np.asarray(inputs["bk"], np.float32)
    Wv = np.asarray(inputs["Wv"], np.float32)
    bv = np.asarray(inputs["bv"], np.float32)

    import ml_dtypes

    bf16 = ml_dtypes.bfloat16
    scale = 1.0 / np.sqrt(np.float32(DH))
    shared = {
        "wqT": np.ascontiguousarray(Wq.T).astype(bf16),
        "wkT": np.ascontiguousarray(Wk.T * scale).astype(bf16),
        "wvT": np.ascontiguousarray(Wv.T).astype(bf16),
        "bqT": np.ascontiguousarray(bq.reshape(NT, 128).T),
        "bkT": np.ascontiguousarray((bk * scale).reshape(NT, 128).T),
        "bvB": np.ascontiguousarray(np.broadcast_to(bv, (128, D))),
        "ones97": np.ones((97, DH), np.float32),
        "vones": np.ones((128, NS, H), bf16),
        "perm": np.eye(128, dtype=np.float32)[
            :, list(range(64, 128)) + list(range(64))
        ].astype(bf16),
    }
    in_maps = []
    for b in range(B):
        m = dict(shared)
        hsTb = np.ascontiguousarray(hs[b].T)
        m["hsT"] = hsTb.astype(bf16)
        m["hsTk"] = (hsTb * click[b][None, :]).astype(bf16)
        m["maskT"] = np.ascontiguousarray(
            np.exp(mask[b, 0, 0].astype(np.float64))
            .astype(np.float32)
            .reshape(NS, 128)
            .T
        )
        in_maps.append(m)
    return in_maps


def run(inputs, trace=False, tmpdir=None):
    """Run on the 8 cores; returns (output [B,S,D], BassKernelResults)."""
    from concourse.bass_utils import run_bass_kernel_spmd

    nc = _get_built()
    in_maps = _prep_in_maps(inputs)
    res = run_bass_kernel_spmd(
        nc, in_maps, list(range(B)), trace=trace, tmpdir=tmpdir
    )
    out = np.empty((B, S, D), np.float32)
    for b in range(B):
        ctxT = np.asarray(res.results[b]["out"], dtype=np.float32)  # [H, DH, S]
        out[b] = ctxT.transpose(2, 0, 1).reshape(S, D)
    return out, res


def kernel(**inputs) -> np.ndarray:
    out, _ = run(inputs)
    return out
